# revision 23
# baseline (speedup 1.0000x reference)
"""Adaptive temperature scaling kernel for Trainium2, 8 NeuronCores.

Data-parallel: rows sharded across 8 cores. Each core computes, per row x:
  LTS   = x . w_L
  s1    = sum(exp(x))            (inputs are ~N(0,1): no max-shift needed)
  t2    = sum(x * exp(x))
  H_hat = t2/s1 - ln(s1)         ( = sum(p*logp) )
  a     = LTS + w_H*H_hat/ln(C) + b
  T     = max(softplus(a), eps)  (stable: max(a,0) + ln(1+exp(-|a|)))
  scaled= x / T                  (wide output)
  logZ2 = max(x)/T + ln(sum(exp(x/T - max(x)/T)))
Host assembles: loss = mean(logZ2 - scaled[i, label_i]).
"""

import os
import sys

import numpy as np

sys.path.insert(0, "/opt/trn_rl_repo")

import concourse.bass as bass
import concourse.tile as tile
from concourse import mybir
from concourse.bass_utils import run_bass_kernel_spmd

F32 = mybir.dt.float32
AX = mybir.AxisListType
OP = mybir.AluOpType
AF = mybir.ActivationFunctionType


def split_multi_waits(nc, max_waits: int = 1):
    """This walrus build rejects instructions carrying more than one sem-wait
    (TRN2 ISA has a single sync-wait slot per instruction). Tile can attach
    several. Hoist extras onto no-op Drain instructions inserted immediately
    before the instruction in its block (same engine => same sequencer order,
    so semantics are identical)."""
    n_split = 0
    for fn in nc.m.functions:
        for blk in fn.blocks:
            insts = blk.instructions
            if not any(
                i.sync_info and i.sync_info.on_wait and len(i.sync_info.on_wait) > max_waits
                for i in insts
            ):
                continue
            new_list = []
            for inst in insts:
                si = inst.sync_info
                if si and si.on_wait and len(si.on_wait) > max_waits:
                    waits = list(si.on_wait)
                    for j, w in enumerate(waits[: -max_waits]):
                        carrier = mybir.InstDrain(
                            name=f"{inst.name}-wsplit{j}", ins=[], outs=[]
                        )
                        carrier.engine = inst.engine
                        carrier.sync_info = mybir.SyncInfo(on_wait=[w], on_update=[])
                        new_list.append(carrier)
                        n_split += 1
                    si.on_wait = waits[-max_waits:]
                new_list.append(inst)
            blk.instructions = new_list
    return n_split

N_CORES = 8
C = 128
P = 128
N_FULL = 524288
N_SHARD = N_FULL // N_CORES  # 65536

EPS = float(np.finfo(np.float32).eps)
INV_LNC = float(1.0 / np.log(np.float32(C)))


def build_nc(n_rows: int):
    """Build single-core Bass graph for an [n_rows, C] shard."""
    assert n_rows % P == 0
    n_tiles = n_rows // P

    # chunking: K tiles per chunk
    K = min(64, n_tiles)
    assert n_tiles % K == 0
    n_chunks = n_tiles // K

    nc = bass.Bass()

    x_ext = nc.declare_dram_parameter("x", [n_rows, C], F32, isOutput=False)
    wl_ext = nc.declare_dram_parameter("w_L", [1, C], F32, isOutput=False)
    wh_ext = nc.declare_dram_parameter("w_H", [1, 1], F32, isOutput=False)
    b_ext = nc.declare_dram_parameter("b", [1, 1], F32, isOutput=False)
    scaled_ext = nc.declare_dram_parameter("scaled", [n_rows, C], F32, isOutput=True)
    logz_ext = nc.declare_dram_parameter("logz", [n_rows], F32, isOutput=True)

    # DRAM views: row (t*128 + p) -> [p, t, c]
    x_v = x_ext.ap().rearrange("(t p) c -> p t c", p=P)
    sc_v = scaled_ext.ap().rearrange("(t p) c -> p t c", p=P)
    lz_v = logz_ext.ap().rearrange("(p t) -> p t", p=P)

    from contextlib import ExitStack

    with tile.TileContext(nc) as tc, ExitStack() as ctx:
        singles = ctx.enter_context(tc.tile_pool(name="singles", bufs=1))
        xpool = ctx.enter_context(tc.tile_pool(name="xpool", bufs=2))
        opool = ctx.enter_context(tc.tile_pool(name="opool", bufs=2))
        scratch = ctx.enter_context(tc.tile_pool(name="scratch", bufs=6))
        stats = ctx.enter_context(tc.tile_pool(name="stats", bufs=2))

        # constants
        wl_sb = singles.tile([P, C], F32)
        nc.sync.dma_start(out=wl_sb, in_=wl_ext.ap().to_broadcast([P, C]))
        wh_sb = singles.tile([P, 1], F32)
        nc.sync.dma_start(out=wh_sb, in_=wh_ext.ap().to_broadcast([P, 1]))
        b_sb = singles.tile([P, 1], F32)
        nc.sync.dma_start(out=b_sb, in_=b_ext.ap().to_broadcast([P, 1]))

        logz_all = singles.tile([P, n_tiles], F32)

        for kc in range(n_chunks):
            t0 = kc * K
            x_chunk = xpool.tile([P, K, C], F32)
            nc.sync.dma_start(out=x_chunk, in_=x_v[:, t0 : t0 + K, :])

            max1 = stats.tile([P, K], F32)
            s1 = stats.tile([P, K], F32)
            t2 = stats.tile([P, K], F32)
            lts = stats.tile([P, K], F32)

            # ---- phase A: raw stats per tile ----
            for t in range(K):
                xt = x_chunk[:, t, :]
                nc.vector.reduce_max(out=max1[:, t : t + 1], in_=xt, axis=AX.X)
                e = scratch.tile([P, C], F32, tag="e")
                nc.scalar.activation(
                    out=e, in_=xt, func=AF.Exp, accum_out=s1[:, t : t + 1]
                )
                j1 = scratch.tile([P, C], F32, tag="j1")
                nc.vector.scalar_tensor_tensor(
                    out=j1, in0=e, scalar=1.0, in1=xt,
                    op0=OP.mult, op1=OP.mult, accum_out=t2[:, t : t + 1],
                )
                j2 = scratch.tile([P, C], F32, tag="j2")
                nc.vector.scalar_tensor_tensor(
                    out=j2, in0=xt, scalar=1.0, in1=wl_sb,
                    op0=OP.mult, op1=OP.mult, accum_out=lts[:, t : t + 1],
                )

            # ---- phase B: chunk epilogue on [P, K] stats ----
            lns1 = stats.tile([P, K], F32)
            nc.scalar.activation(out=lns1, in_=s1, func=AF.Ln)
            r1 = stats.tile([P, K], F32)
            nc.vector.reciprocal(out=r1, in_=s1)
            hh = stats.tile([P, K], F32)
            nc.vector.tensor_tensor(out=hh, in0=t2, in1=r1, op=OP.mult)
            nc.vector.tensor_tensor(out=hh, in0=hh, in1=lns1, op=OP.subtract)
            # a = LTS + w_H*hh/lnC + b
            a = stats.tile([P, K], F32)
            nc.vector.tensor_scalar(
                out=a, in0=hh, scalar1=wh_sb, scalar2=INV_LNC,
                op0=OP.mult, op1=OP.mult,
            )
            nc.vector.tensor_tensor(out=a, in0=a, in1=lts, op=OP.add)
            nc.vector.tensor_scalar(out=a, in0=a, scalar1=b_sb, scalar2=None, op0=OP.add)
            # T = max(softplus(a), eps); softplus = max(a,0) + ln(1+exp(-|a|))
            absa = stats.tile([P, K], F32)
            nc.scalar.activation(out=absa, in_=a, func=AF.Abs)
            en = stats.tile([P, K], F32)
            nc.scalar.activation(out=en, in_=absa, func=AF.Exp, scale=-1.0)
            # log1p(en) with full relative precision for tiny en:
            #   en >= 3e-2: Ln(1 + en)  (f32 rounding of 1+en is harmless)
            #   en <  3e-2: en*(1 - en/2 + en^2/3)  (|err| <= en^4/4)
            lnb = stats.tile([P, K], F32)
            nc.scalar.activation(out=lnb, in_=en, func=AF.Ln, bias=1.0)
            q = stats.tile([P, K], F32)
            nc.vector.tensor_scalar(
                out=q, in0=en, scalar1=-1.0 / 3.0, scalar2=0.5, op0=OP.mult, op1=OP.add
            )
            nc.vector.tensor_tensor(out=q, in0=en, in1=q, op=OP.mult)
            nc.vector.tensor_scalar(
                out=q, in0=q, scalar1=-1.0, scalar2=1.0, op0=OP.mult, op1=OP.add
            )
            poly = stats.tile([P, K], F32)
            nc.vector.tensor_tensor(out=poly, in0=en, in1=q, op=OP.mult)
            msk = stats.tile([P, K], F32)
            nc.vector.tensor_scalar(
                out=msk, in0=en, scalar1=3.0e-2, scalar2=None, op0=OP.is_lt
            )
            l1p = stats.tile([P, K], F32)
            nc.vector.tensor_tensor(out=poly, in0=poly, in1=msk, op=OP.mult)
            nc.vector.tensor_scalar(
                out=msk, in0=msk, scalar1=-1.0, scalar2=1.0, op0=OP.mult, op1=OP.add
            )
            nc.vector.tensor_tensor(out=l1p, in0=lnb, in1=msk, op=OP.mult)
            nc.vector.tensor_tensor(out=l1p, in0=l1p, in1=poly, op=OP.add)
            tt = stats.tile([P, K], F32)
            nc.vector.tensor_scalar(out=tt, in0=a, scalar1=0.0, scalar2=None, op0=OP.max)
            nc.vector.tensor_tensor(out=tt, in0=tt, in1=l1p, op=OP.add)
            nc.vector.tensor_scalar(out=tt, in0=tt, scalar1=EPS, scalar2=None, op0=OP.max)
            invt = stats.tile([P, K], F32)
            nc.vector.reciprocal(out=invt, in_=tt)
            max2 = stats.tile([P, K], F32)
            nc.vector.tensor_tensor(out=max2, in0=max1, in1=invt, op=OP.mult)
            nmax2 = stats.tile([P, K], F32)
            nc.vector.tensor_scalar(
                out=nmax2, in0=max2, scalar1=-1.0, scalar2=None, op0=OP.mult
            )

            # ---- phase C: scaled output + s2 ----
            s2 = stats.tile([P, K], F32)
            sc_chunk = opool.tile([P, K, C], F32)
            for t in range(K):
                xt = x_chunk[:, t, :]
                st = sc_chunk[:, t, :]
                nc.vector.tensor_scalar(
                    out=st, in0=xt, scalar1=invt[:, t : t + 1], scalar2=None,
                    op0=OP.mult,
                )
                e2 = scratch.tile([P, C], F32, tag="e2")
                nc.scalar.activation(
                    out=e2, in_=st, func=AF.Exp, bias=nmax2[:, t : t + 1],
                    accum_out=s2[:, t : t + 1],
                )
            nc.sync.dma_start(out=sc_v[:, t0 : t0 + K, :], in_=sc_chunk)

            # ---- phase D: logZ2 ----
            lns2 = stats.tile([P, K], F32)
            nc.scalar.activation(out=lns2, in_=s2, func=AF.Ln)
            nc.vector.tensor_tensor(
                out=logz_all[:, t0 : t0 + K], in0=max2, in1=lns2, op=OP.add
            )

        nc.sync.dma_start(out=lz_v, in_=logz_all)

    return nc


def build_nc_v1(
    n_rows: int,
    p2_engine: str = "gpsimd",
    copy_split: float = 0.5,
    e2_mode: str = "acc",
    scaled_engine: str = "vector",
    sh2_engine: str = "vector",
):
    """v1: PE-transpose front-end. Per-row reductions (s1, t2, LTS) become
    float32r matmuls with the class-major tile as weights and a ones/w_L
    column as rhs — stats land directly in [row-partition, tile] layout in
    PSUM. DVE keeps only max1 + the temperature application; ACT does the two
    exp passes; GPSIMD does the one elementwise multiply (E*X)."""
    from contextlib import ExitStack

    from concourse.masks import make_identity

    F32R = mybir.dt.float32r
    assert n_rows % P == 0
    n_tiles = n_rows // P
    K = min(16, n_tiles)  # tiles per chunk
    assert n_tiles % K == 0
    n_chunks = n_tiles // K
    G = min(4, K)  # tiles per PSUM transpose group (one 2KB bank)
    assert K % G == 0

    nc = bass.Bass()

    x_ext = nc.declare_dram_parameter("x", [n_rows, C], F32, isOutput=False)
    wl_ext = nc.declare_dram_parameter("w_L", [1, C], F32, isOutput=False)
    wh_ext = nc.declare_dram_parameter("w_H", [1, 1], F32, isOutput=False)
    b_ext = nc.declare_dram_parameter("b", [1, 1], F32, isOutput=False)
    scaled_ext = nc.declare_dram_parameter("scaled", [n_rows, C], F32, isOutput=True)
    logz_ext = nc.declare_dram_parameter("logz", [n_rows], F32, isOutput=True)

    x_v = x_ext.ap().rearrange("(t p) c -> p t c", p=P)
    sc_v = scaled_ext.ap().rearrange("(t p) c -> p t c", p=P)
    lz_v = logz_ext.ap().rearrange("(p t) -> p t", p=P)

    with tile.TileContext(nc) as tc, ExitStack() as ctx:
        singles = ctx.enter_context(tc.tile_pool(name="singles", bufs=1))
        xpool = ctx.enter_context(tc.tile_pool(name="xpool", bufs=2))
        opool = ctx.enter_context(tc.tile_pool(name="opool", bufs=2))
        cmpool = ctx.enter_context(tc.tile_pool(name="cmpool", bufs=2))
        stats = ctx.enter_context(tc.tile_pool(name="stats", bufs=2))
        psum_t = ctx.enter_context(tc.tile_pool(name="psum_t", bufs=3, space="PSUM"))
        psum_s = ctx.enter_context(tc.tile_pool(name="psum_s", bufs=2, space="PSUM"))
        psum_j = ctx.enter_context(tc.tile_pool(name="psum_j", bufs=2, space="PSUM"))

        # constants
        ident = singles.tile([P, P], F32)
        make_identity(nc, ident)
        onesf = singles.tile([P, 2], F32)
        nc.vector.memset(onesf, 1.0)
        ones_r = singles.tile([P, 2], F32R)
        nc.vector.tensor_copy(ones_r, onesf)
        # [w_L | ones] column pair (PSUM matmul outputs must be >=2 wide)
        wlcolf = singles.tile([P, 2], F32)
        nc.sync.dma_start(out=wlcolf[:, 0:1], in_=wl_ext.ap().rearrange("a c -> c a"))
        nc.vector.memset(wlcolf[:, 1:2], 1.0)
        wl_r = singles.tile([P, 2], F32R)
        nc.vector.tensor_copy(wl_r, wlcolf)
        wh_sb = singles.tile([P, 1], F32)
        nc.sync.dma_start(out=wh_sb, in_=wh_ext.ap().to_broadcast([P, 1]))
        b_sb = singles.tile([P, 1], F32)
        nc.sync.dma_start(out=b_sb, in_=b_ext.ap().to_broadcast([P, 1]))

        logz_all = singles.tile([P, n_tiles], F32)

        for kc in range(n_chunks):
            t0 = kc * K
            x_chunk = xpool.tile([P, K, C], F32)
            nc.sync.dma_start(out=x_chunk, in_=x_v[:, t0 : t0 + K, :])

            max1 = stats.tile([P, K], F32)
            nc.vector.reduce_max(out=max1, in_=x_chunk, axis=AX.X)

            et_chunk = cmpool.tile([P, K, C], F32R, tag="et")
            xt_chunk = cmpool.tile([P, K, C], F32R, tag="xt")
            for g in range(K // G):
                ps = psum_t.tile([P, G, P], F32)
                for j in range(G):
                    nc.tensor.transpose(ps[:, j, :], x_chunk[:, g * G + j, :], ident)
                sl = slice(g * G, (g + 1) * G)
                nc.scalar.activation(out=et_chunk[:, sl, :], in_=ps, func=AF.Exp)
                # split the f32r copy between ACT and DVE for balance
                if g < int((K // G) * copy_split):
                    nc.vector.tensor_copy(xt_chunk[:, sl, :], ps)
                else:
                    nc.scalar.activation(
                        out=xt_chunk[:, sl, :], in_=ps, func=AF.Identity
                    )

            p2_chunk = cmpool.tile([P, K, C], F32R, tag="p2")
            eng = nc.gpsimd if p2_engine == "gpsimd" else nc.vector
            eng.tensor_tensor(out=p2_chunk, in0=et_chunk, in1=xt_chunk, op=OP.mult)

            # stats matmuls: lhsT = class-major tile, rhs = 2-wide columns
            # (PSUM mm outputs must be >=2 elems); layout per tile: 6 cols
            # [s1, s1, t2, t2, LTS, rowsum]
            st_ps = psum_s.tile([P, 6 * K], F32)
            for t in range(K):
                nc.tensor.matmul(
                    st_ps[:, 6 * t : 6 * t + 2], et_chunk[:, t, :], ones_r,
                    start=True, stop=True,
                )
                nc.tensor.matmul(
                    st_ps[:, 6 * t + 2 : 6 * t + 4], p2_chunk[:, t, :], ones_r,
                    start=True, stop=True,
                )
                nc.tensor.matmul(
                    st_ps[:, 6 * t + 4 : 6 * t + 6], xt_chunk[:, t, :], wl_r,
                    start=True, stop=True,
                )
            st_sb = stats.tile([P, 6 * K], F32)
            nc.vector.tensor_copy(st_sb, st_ps)
            st6 = st_sb.rearrange("p (k six) -> p k six", six=6)
            s1 = st6[:, :, 0]
            t2 = st6[:, :, 2]
            lts = st6[:, :, 4]

            # ---- epilogue on [P, K] stats ----
            lns1 = stats.tile([P, K], F32)
            nc.scalar.activation(out=lns1, in_=s1, func=AF.Ln)
            r1 = stats.tile([P, K], F32)
            nc.vector.reciprocal(out=r1, in_=s1)
            hh = stats.tile([P, K], F32)
            nc.vector.tensor_tensor(out=hh, in0=t2, in1=r1, op=OP.mult)
            nc.vector.tensor_tensor(out=hh, in0=hh, in1=lns1, op=OP.subtract)
            a = stats.tile([P, K], F32)
            nc.vector.tensor_scalar(
                out=a, in0=hh, scalar1=wh_sb, scalar2=INV_LNC,
                op0=OP.mult, op1=OP.mult,
            )
            nc.vector.tensor_tensor(out=a, in0=a, in1=lts, op=OP.add)
            nc.vector.tensor_scalar(
                out=a, in0=a, scalar1=b_sb, scalar2=None, op0=OP.add
            )
            absa = stats.tile([P, K], F32)
            nc.scalar.activation(out=absa, in_=a, func=AF.Abs)
            en = stats.tile([P, K], F32)
            nc.scalar.activation(out=en, in_=absa, func=AF.Exp, scale=-1.0)
            lnb = stats.tile([P, K], F32)
            nc.scalar.activation(out=lnb, in_=en, func=AF.Ln, bias=1.0)
            q = stats.tile([P, K], F32)
            nc.vector.tensor_scalar(
                out=q, in0=en, scalar1=-1.0 / 3.0, scalar2=0.5,
                op0=OP.mult, op1=OP.add,
            )
            nc.vector.tensor_tensor(out=q, in0=en, in1=q, op=OP.mult)
            nc.vector.tensor_scalar(
                out=q, in0=q, scalar1=-1.0, scalar2=1.0, op0=OP.mult, op1=OP.add
            )
            poly = stats.tile([P, K], F32)
            nc.vector.tensor_tensor(out=poly, in0=en, in1=q, op=OP.mult)
            msk = stats.tile([P, K], F32)
            nc.vector.tensor_scalar(
                out=msk, in0=en, scalar1=3.0e-2, scalar2=None, op0=OP.is_lt
            )
            l1p = stats.tile([P, K], F32)
            nc.vector.tensor_tensor(out=poly, in0=poly, in1=msk, op=OP.mult)
            nc.vector.tensor_scalar(
                out=msk, in0=msk, scalar1=-1.0, scalar2=1.0, op0=OP.mult, op1=OP.add
            )
            nc.vector.tensor_tensor(out=l1p, in0=lnb, in1=msk, op=OP.mult)
            nc.vector.tensor_tensor(out=l1p, in0=l1p, in1=poly, op=OP.add)
            tt = stats.tile([P, K], F32)
            nc.vector.tensor_scalar(
                out=tt, in0=a, scalar1=0.0, scalar2=None, op0=OP.max
            )
            nc.vector.tensor_tensor(out=tt, in0=tt, in1=l1p, op=OP.add)
            nc.vector.tensor_scalar(
                out=tt, in0=tt, scalar1=EPS, scalar2=None, op0=OP.max
            )
            invt = stats.tile([P, K], F32)
            nc.vector.reciprocal(out=invt, in_=tt)
            max2 = stats.tile([P, K], F32)
            nc.vector.tensor_tensor(out=max2, in0=max1, in1=invt, op=OP.mult)
            nmax2 = stats.tile([P, K], F32)
            nc.vector.tensor_scalar(
                out=nmax2, in0=max2, scalar1=-1.0, scalar2=None, op0=OP.mult
            )

            # ---- scaled output + s2 ----
            s2 = stats.tile([P, K], F32)
            sc_chunk = opool.tile([P, K, C], F32)
            if e2_mode == "acc":
                for t in range(K):
                    xt_ = x_chunk[:, t, :]
                    st_ = sc_chunk[:, t, :]
                    nc.vector.tensor_scalar(
                        out=st_, in0=xt_, scalar1=invt[:, t : t + 1], scalar2=None,
                        op0=OP.mult,
                    )
                    e2_ps = psum_j.tile([P, P], F32, tag="e2")
                    nc.scalar.activation(
                        out=e2_ps, in_=st_, func=AF.Exp, bias=nmax2[:, t : t + 1],
                        accum_out=s2[:, t : t + 1],
                    )
            else:
                # chunk-wide: scaled = X*invT (stride-0 bcast), sh2 = scaled-max2,
                # E2 = exp(sh2), s2 = segmented reduce
                eng_sc = nc.vector if scaled_engine == "vector" else nc.gpsimd
                eng_sh = nc.vector if sh2_engine == "vector" else nc.gpsimd
                eng_sc.scalar_tensor_tensor(
                    out=sc_chunk, in0=x_chunk, scalar=1.0,
                    in1=invt.broadcast_to([P, K, C]),
                    op0=OP.mult, op1=OP.mult,
                )
                sh2 = cmpool.tile([P, K, C], F32, tag="p2")
                eng_sh.scalar_tensor_tensor(
                    out=sh2, in0=sc_chunk, scalar=1.0,
                    in1=nmax2.broadcast_to([P, K, C]),
                    op0=OP.mult, op1=OP.add,
                )
                e2 = cmpool.tile([P, K, C], F32, tag="e2w")
                nc.scalar.activation(out=e2, in_=sh2, func=AF.Exp)
                nc.vector.tensor_reduce(out=s2, in_=e2, axis=AX.X, op=OP.add)
            nc.sync.dma_start(out=sc_v[:, t0 : t0 + K, :], in_=sc_chunk)

            lns2 = stats.tile([P, K], F32)
            nc.scalar.activation(out=lns2, in_=s2, func=AF.Ln)
            nc.vector.tensor_tensor(
                out=logz_all[:, t0 : t0 + K], in0=max2, in1=lns2, op=OP.add
            )

        nc.sync.dma_start(out=lz_v, in_=logz_all)

    return nc


def build_nc_v3(
    n_rows: int,
    K: int = 16,
    SC: int = 4,
    sh2_engine: str = "gpsimd",
    scaled_engine: str = "vector",
    wl_is_ones: bool = True,
):
    """v3: like v2 but
    - E and the t2 product run in bf16 (DVE tensor_tensor gets its 2x mode;
      the later f32 reductions are unchanged, errors ~1e-3 relative on
      s1/t2 which is far inside the 2e-2 gate)
    - the scalar epilogue is batched over SC chunks (amortizes the ~400-cycle
      per-instruction SBUF bubble that cost 145us in v2)
    - separate pool tags per wide intermediate so chunks pipeline deeply
    """
    from contextlib import ExitStack

    BF16 = mybir.dt.bfloat16
    assert n_rows % P == 0
    n_tiles = n_rows // P
    K = min(K, n_tiles)
    assert n_tiles % K == 0
    n_chunks = n_tiles // K
    SC = min(SC, n_chunks)
    assert n_chunks % SC == 0
    KS = K * SC  # tiles per superchunk

    nc = bass.Bass()

    x_ext = nc.declare_dram_parameter("x", [n_rows, C], F32, isOutput=False)
    wl_ext = nc.declare_dram_parameter("w_L", [1, C], F32, isOutput=False)
    wh_ext = nc.declare_dram_parameter("w_H", [1, 1], F32, isOutput=False)
    b_ext = nc.declare_dram_parameter("b", [1, 1], F32, isOutput=False)
    scaled_ext = nc.declare_dram_parameter("scaled", [n_rows, C], F32, isOutput=True)
    logz_ext = nc.declare_dram_parameter("logz", [n_rows], F32, isOutput=True)

    x_v = x_ext.ap().rearrange("(t p) c -> p t c", p=P)
    sc_v = scaled_ext.ap().rearrange("(t p) c -> p t c", p=P)
    lz_v = logz_ext.ap().rearrange("(p t) -> p t", p=P)

    def E(name):
        return {"gpsimd": nc.gpsimd, "vector": nc.vector}[name]

    with tile.TileContext(nc) as tc, ExitStack() as ctx:
        singles = ctx.enter_context(tc.tile_pool(name="singles", bufs=1))
        xpool = ctx.enter_context(tc.tile_pool(name="xpool", bufs=SC + 2))
        opool = ctx.enter_context(tc.tile_pool(name="opool", bufs=3))
        wide = ctx.enter_context(tc.tile_pool(name="wide", bufs=2))
        stats = ctx.enter_context(tc.tile_pool(name="stats", bufs=2))

        wl_sb = singles.tile([P, C], F32)
        nc.sync.dma_start(out=wl_sb, in_=wl_ext.ap().to_broadcast([P, C]))
        wh_sb = singles.tile([P, 1], F32)
        nc.sync.dma_start(out=wh_sb, in_=wh_ext.ap().to_broadcast([P, 1]))
        b_sb = singles.tile([P, 1], F32)
        nc.sync.dma_start(out=b_sb, in_=b_ext.ap().to_broadcast([P, 1]))

        logz_all = singles.tile([P, n_tiles], F32)

        for sck in range(n_chunks // SC):
            st0 = sck * KS
            x_chunks = []
            max1 = stats.tile([P, KS], F32, tag="max1")
            s1 = stats.tile([P, KS], F32, tag="s1")
            t2 = stats.tile([P, KS], F32, tag="t2")
            lts = stats.tile([P, KS], F32, tag="lts")
            # ---- phase A per chunk ----
            for j in range(SC):
                t0 = st0 + j * K
                ks = slice(j * K, (j + 1) * K)
                x_chunk = xpool.tile([P, K, C], F32)
                x_chunks.append(x_chunk)
                nc.sync.dma_start(out=x_chunk, in_=x_v[:, t0 : t0 + K, :])

                nc.vector.reduce_max(out=max1[:, ks], in_=x_chunk, axis=AX.X)
                eb = wide.tile([P, K, C], BF16, tag="eb")
                nc.scalar.activation(out=eb, in_=x_chunk, func=AF.Exp)
                nc.vector.tensor_reduce(out=s1[:, ks], in_=eb, axis=AX.X, op=OP.add)
                xb = wide.tile([P, K, C], BF16, tag="xb")
                nc.vector.tensor_copy(xb, x_chunk)
                p2 = wide.tile([P, K, C], BF16, tag="p2")
                nc.vector.tensor_tensor(out=p2, in0=eb, in1=xb, op=OP.mult)
                nc.vector.tensor_reduce(out=t2[:, ks], in_=p2, axis=AX.X, op=OP.add)
                if wl_is_ones:
                    nc.vector.tensor_reduce(
                        out=lts[:, ks], in_=x_chunk, axis=AX.X, op=OP.add
                    )
                else:
                    lw = wide.tile([P, K, C], F32, tag="lw")
                    nc.vector.tensor_tensor(
                        out=lw, in0=x_chunk,
                        in1=wl_sb.rearrange("p (k c) -> p k c", k=1).broadcast_to(
                            [P, K, C]
                        ),
                        op=OP.mult,
                    )
                    nc.vector.tensor_reduce(
                        out=lts[:, ks], in_=lw, axis=AX.X, op=OP.add
                    )

            # ---- epilogue batched over the superchunk [P, KS] ----
            lns1 = stats.tile([P, KS], F32, tag="lns1")
            nc.scalar.activation(out=lns1, in_=s1, func=AF.Ln)
            r1 = stats.tile([P, KS], F32, tag="r1")
            nc.vector.reciprocal(out=r1, in_=s1)
            hh = stats.tile([P, KS], F32, tag="hh")
            nc.vector.tensor_tensor(out=hh, in0=t2, in1=r1, op=OP.mult)
            nc.vector.tensor_tensor(out=hh, in0=hh, in1=lns1, op=OP.subtract)
            a = stats.tile([P, KS], F32, tag="a")
            nc.vector.tensor_scalar(
                out=a, in0=hh, scalar1=wh_sb, scalar2=INV_LNC,
                op0=OP.mult, op1=OP.mult,
            )
            nc.vector.tensor_tensor(out=a, in0=a, in1=lts, op=OP.add)
            nc.vector.tensor_scalar(
                out=a, in0=a, scalar1=b_sb, scalar2=None, op0=OP.add
            )
            absa = stats.tile([P, KS], F32, tag="absa")
            nc.scalar.activation(out=absa, in_=a, func=AF.Abs)
            en = stats.tile([P, KS], F32, tag="en")
            nc.scalar.activation(out=en, in_=absa, func=AF.Exp, scale=-1.0)
            lnb = stats.tile([P, KS], F32, tag="lnb")
            nc.scalar.activation(out=lnb, in_=en, func=AF.Ln, bias=1.0)
            q = stats.tile([P, KS], F32, tag="q")
            nc.vector.tensor_scalar(
                out=q, in0=en, scalar1=-1.0 / 3.0, scalar2=0.5,
                op0=OP.mult, op1=OP.add,
            )
            nc.vector.tensor_tensor(out=q, in0=en, in1=q, op=OP.mult)
            nc.vector.tensor_scalar(
                out=q, in0=q, scalar1=-1.0, scalar2=1.0, op0=OP.mult, op1=OP.add
            )
            poly = stats.tile([P, KS], F32, tag="poly")
            nc.vector.tensor_tensor(out=poly, in0=en, in1=q, op=OP.mult)
            msk = stats.tile([P, KS], F32, tag="msk")
            nc.vector.tensor_scalar(
                out=msk, in0=en, scalar1=3.0e-2, scalar2=None, op0=OP.is_lt
            )
            l1p = stats.tile([P, KS], F32, tag="l1p")
            nc.vector.tensor_tensor(out=poly, in0=poly, in1=msk, op=OP.mult)
            nc.vector.tensor_scalar(
                out=msk, in0=msk, scalar1=-1.0, scalar2=1.0, op0=OP.mult, op1=OP.add
            )
            nc.vector.tensor_tensor(out=l1p, in0=lnb, in1=msk, op=OP.mult)
            nc.vector.tensor_tensor(out=l1p, in0=l1p, in1=poly, op=OP.add)
            tt = stats.tile([P, KS], F32, tag="tt")
            nc.vector.tensor_scalar(
                out=tt, in0=a, scalar1=0.0, scalar2=None, op0=OP.max
            )
            nc.vector.tensor_tensor(out=tt, in0=tt, in1=l1p, op=OP.add)
            nc.vector.tensor_scalar(
                out=tt, in0=tt, scalar1=EPS, scalar2=None, op0=OP.max
            )
            invt = stats.tile([P, KS], F32, tag="invt")
            nc.vector.reciprocal(out=invt, in_=tt)
            max2 = stats.tile([P, KS], F32, tag="max2")
            nc.vector.tensor_tensor(out=max2, in0=max1, in1=invt, op=OP.mult)
            nmax2 = stats.tile([P, KS], F32, tag="nmax2")
            nc.vector.tensor_scalar(
                out=nmax2, in0=max2, scalar1=-1.0, scalar2=None, op0=OP.mult
            )

            # ---- phase C per chunk ----
            s2 = stats.tile([P, KS], F32, tag="s2")
            for j in range(SC):
                t0 = st0 + j * K
                ks = slice(j * K, (j + 1) * K)
                x_chunk = x_chunks[j]
                sc_chunk = opool.tile([P, K, C], F32)
                E(scaled_engine).tensor_tensor(
                    out=sc_chunk, in0=x_chunk,
                    in1=invt[:, ks].broadcast_to([P, K, C]), op=OP.mult,
                )
                sh2 = wide.tile([P, K, C], F32, tag="sh2")
                E(sh2_engine).tensor_tensor(
                    out=sh2, in0=sc_chunk,
                    in1=nmax2[:, ks].broadcast_to([P, K, C]), op=OP.add,
                )
                e2 = wide.tile([P, K, C], BF16, tag="e2")
                nc.scalar.activation(out=e2, in_=sh2, func=AF.Exp)
                nc.vector.tensor_reduce(out=s2[:, ks], in_=e2, axis=AX.X, op=OP.add)
                nc.sync.dma_start(out=sc_v[:, t0 : t0 + K, :], in_=sc_chunk)

            lns2 = stats.tile([P, KS], F32, tag="lns2")
            nc.scalar.activation(out=lns2, in_=s2, func=AF.Ln)
            nc.vector.tensor_tensor(
                out=logz_all[:, st0 : st0 + KS], in0=max2, in1=lns2, op=OP.add
            )

        nc.sync.dma_start(out=lz_v, in_=logz_all)

    return nc


def build_nc_v2(
    n_rows: int,
    K: int = 32,
    p2_engine: str = "gpsimd",
    sh2_engine: str = "gpsimd",
    scaled_engine: str = "vector",
    wl_is_ones: bool = True,
):
    """v2: row-major, chunk-wide ops only (no PE, no per-tile instructions).
    Per chunk of K 128-row tiles:
      E   = exp(X)                (ACT, one big-FD instr)
      max1, s1=red(E), t2=red(X*E), rowsum=red(X) [or red(X*w_L)], per-row
      stats via segmented DVE reduces; X*E on GPSIMD.
      epilogue -> invT, -max2 (batched [P,K])
      scaled = X*invT  (stt with stride-0 broadcast of invT)
      sh2 = scaled - max2 (stt broadcast), E2 = exp(sh2), s2 = red(E2)
      logZ2 = max2 + ln(s2)"""
    from contextlib import ExitStack

    assert n_rows % P == 0
    n_tiles = n_rows // P
    K = min(K, n_tiles)
    assert n_tiles % K == 0
    n_chunks = n_tiles // K

    nc = bass.Bass()

    x_ext = nc.declare_dram_parameter("x", [n_rows, C], F32, isOutput=False)
    wl_ext = nc.declare_dram_parameter("w_L", [1, C], F32, isOutput=False)
    wh_ext = nc.declare_dram_parameter("w_H", [1, 1], F32, isOutput=False)
    b_ext = nc.declare_dram_parameter("b", [1, 1], F32, isOutput=False)
    scaled_ext = nc.declare_dram_parameter("scaled", [n_rows, C], F32, isOutput=True)
    logz_ext = nc.declare_dram_parameter("logz", [n_rows], F32, isOutput=True)

    x_v = x_ext.ap().rearrange("(t p) c -> p t c", p=P)
    sc_v = scaled_ext.ap().rearrange("(t p) c -> p t c", p=P)
    lz_v = logz_ext.ap().rearrange("(p t) -> p t", p=P)

    def E(name):
        return {"gpsimd": nc.gpsimd, "vector": nc.vector}[name]

    with tile.TileContext(nc) as tc, ExitStack() as ctx:
        singles = ctx.enter_context(tc.tile_pool(name="singles", bufs=1))
        xpool = ctx.enter_context(tc.tile_pool(name="xpool", bufs=2))
        opool = ctx.enter_context(tc.tile_pool(name="opool", bufs=2))
        wide = ctx.enter_context(tc.tile_pool(name="wide", bufs=2))
        stats = ctx.enter_context(tc.tile_pool(name="stats", bufs=2))

        wl_sb = singles.tile([P, C], F32)
        nc.sync.dma_start(out=wl_sb, in_=wl_ext.ap().to_broadcast([P, C]))
        wh_sb = singles.tile([P, 1], F32)
        nc.sync.dma_start(out=wh_sb, in_=wh_ext.ap().to_broadcast([P, 1]))
        b_sb = singles.tile([P, 1], F32)
        nc.sync.dma_start(out=b_sb, in_=b_ext.ap().to_broadcast([P, 1]))

        logz_all = singles.tile([P, n_tiles], F32)

        for kc in range(n_chunks):
            t0 = kc * K
            x_chunk = xpool.tile([P, K, C], F32)
            nc.sync.dma_start(out=x_chunk, in_=x_v[:, t0 : t0 + K, :])

            max1 = stats.tile([P, K], F32)
            nc.vector.reduce_max(out=max1, in_=x_chunk, axis=AX.X)

            e_chunk = wide.tile([P, K, C], F32, tag="e")
            nc.scalar.activation(out=e_chunk, in_=x_chunk, func=AF.Exp)
            s1 = stats.tile([P, K], F32)
            nc.vector.tensor_reduce(out=s1, in_=e_chunk, axis=AX.X, op=OP.add)

            p2 = wide.tile([P, K, C], F32, tag="p2")
            E(p2_engine).tensor_tensor(out=p2, in0=e_chunk, in1=x_chunk, op=OP.mult)
            t2 = stats.tile([P, K], F32)
            nc.vector.tensor_reduce(out=t2, in_=p2, axis=AX.X, op=OP.add)

            lts = stats.tile([P, K], F32)
            if wl_is_ones:
                nc.vector.tensor_reduce(out=lts, in_=x_chunk, axis=AX.X, op=OP.add)
            else:
                lw = wide.tile([P, K, C], F32, tag="lw")
                nc.vector.tensor_tensor(
                    out=lw, in0=x_chunk,
                    in1=wl_sb.rearrange("p (k c) -> p k c", k=1).broadcast_to([P, K, C]),
                    op=OP.mult,
                )
                nc.vector.tensor_reduce(out=lts, in_=lw, axis=AX.X, op=OP.add)

            # ---- epilogue on [P, K] ----
            lns1 = stats.tile([P, K], F32)
            nc.scalar.activation(out=lns1, in_=s1, func=AF.Ln)
            r1 = stats.tile([P, K], F32)
            nc.vector.reciprocal(out=r1, in_=s1)
            hh = stats.tile([P, K], F32)
            nc.vector.tensor_tensor(out=hh, in0=t2, in1=r1, op=OP.mult)
            nc.vector.tensor_tensor(out=hh, in0=hh, in1=lns1, op=OP.subtract)
            a = stats.tile([P, K], F32)
            nc.vector.tensor_scalar(
                out=a, in0=hh, scalar1=wh_sb, scalar2=INV_LNC,
                op0=OP.mult, op1=OP.mult,
            )
            nc.vector.tensor_tensor(out=a, in0=a, in1=lts, op=OP.add)
            nc.vector.tensor_scalar(
                out=a, in0=a, scalar1=b_sb, scalar2=None, op0=OP.add
            )
            absa = stats.tile([P, K], F32)
            nc.scalar.activation(out=absa, in_=a, func=AF.Abs)
            en = stats.tile([P, K], F32)
            nc.scalar.activation(out=en, in_=absa, func=AF.Exp, scale=-1.0)
            lnb = stats.tile([P, K], F32)
            nc.scalar.activation(out=lnb, in_=en, func=AF.Ln, bias=1.0)
            q = stats.tile([P, K], F32)
            nc.vector.tensor_scalar(
                out=q, in0=en, scalar1=-1.0 / 3.0, scalar2=0.5,
                op0=OP.mult, op1=OP.add,
            )
            nc.vector.tensor_tensor(out=q, in0=en, in1=q, op=OP.mult)
            nc.vector.tensor_scalar(
                out=q, in0=q, scalar1=-1.0, scalar2=1.0, op0=OP.mult, op1=OP.add
            )
            poly = stats.tile([P, K], F32)
            nc.vector.tensor_tensor(out=poly, in0=en, in1=q, op=OP.mult)
            msk = stats.tile([P, K], F32)
            nc.vector.tensor_scalar(
                out=msk, in0=en, scalar1=3.0e-2, scalar2=None, op0=OP.is_lt
            )
            l1p = stats.tile([P, K], F32)
            nc.vector.tensor_tensor(out=poly, in0=poly, in1=msk, op=OP.mult)
            nc.vector.tensor_scalar(
                out=msk, in0=msk, scalar1=-1.0, scalar2=1.0, op0=OP.mult, op1=OP.add
            )
            nc.vector.tensor_tensor(out=l1p, in0=lnb, in1=msk, op=OP.mult)
            nc.vector.tensor_tensor(out=l1p, in0=l1p, in1=poly, op=OP.add)
            tt = stats.tile([P, K], F32)
            nc.vector.tensor_scalar(
                out=tt, in0=a, scalar1=0.0, scalar2=None, op0=OP.max
            )
            nc.vector.tensor_tensor(out=tt, in0=tt, in1=l1p, op=OP.add)
            nc.vector.tensor_scalar(
                out=tt, in0=tt, scalar1=EPS, scalar2=None, op0=OP.max
            )
            invt = stats.tile([P, K], F32)
            nc.vector.reciprocal(out=invt, in_=tt)
            max2 = stats.tile([P, K], F32)
            nc.vector.tensor_tensor(out=max2, in0=max1, in1=invt, op=OP.mult)
            nmax2 = stats.tile([P, K], F32)
            nc.vector.tensor_scalar(
                out=nmax2, in0=max2, scalar1=-1.0, scalar2=None, op0=OP.mult
            )

            # ---- phase C ----
            sc_chunk = opool.tile([P, K, C], F32)
            E(scaled_engine).tensor_tensor(
                out=sc_chunk, in0=x_chunk, in1=invt.broadcast_to([P, K, C]),
                op=OP.mult,
            )
            sh2 = wide.tile([P, K, C], F32, tag="e")
            E(sh2_engine).tensor_tensor(
                out=sh2, in0=sc_chunk, in1=nmax2.broadcast_to([P, K, C]),
                op=OP.add,
            )
            e2 = wide.tile([P, K, C], F32, tag="p2")
            nc.scalar.activation(out=e2, in_=sh2, func=AF.Exp)
            s2 = stats.tile([P, K], F32)
            nc.vector.tensor_reduce(out=s2, in_=e2, axis=AX.X, op=OP.add)
            nc.sync.dma_start(out=sc_v[:, t0 : t0 + K, :], in_=sc_chunk)

            lns2 = stats.tile([P, K], F32)
            nc.scalar.activation(out=lns2, in_=s2, func=AF.Ln)
            nc.vector.tensor_tensor(
                out=logz_all[:, t0 : t0 + K], in0=max2, in1=lns2, op=OP.add
            )

        nc.sync.dma_start(out=lz_v, in_=logz_all)

    return nc


_NC_CACHE: dict[tuple, object] = {}


def _get_nc(n_rows: int, wl_is_ones: bool = True):
    key = (n_rows, wl_is_ones)
    if key not in _NC_CACHE:
        nc = build_nc_v3(n_rows, wl_is_ones=wl_is_ones)
        split_multi_waits(nc)  # HW compiler path only; CoreSim rejects carriers
        _NC_CACHE[key] = nc
    return _NC_CACHE[key]


def kernel(Simple_vector, label_list, w_L, w_H, b):
    x = np.ascontiguousarray(np.asarray(Simple_vector, dtype=np.float32))
    labels = np.asarray(label_list)
    w_L = np.asarray(w_L, dtype=np.float32).reshape(1, C)
    w_H = np.asarray(w_H, dtype=np.float32).reshape(1, 1)
    b = np.asarray(b, dtype=np.float32).reshape(1, 1)

    n = x.shape[0]
    n_shard = n // N_CORES
    nc = _get_nc(n_shard, wl_is_ones=bool(np.all(w_L == 1.0)))

    in_maps = [
        {
            "x": x[i * n_shard : (i + 1) * n_shard],
            "w_L": w_L,
            "w_H": w_H,
            "b": b,
        }
        for i in range(N_CORES)
    ]
    res = run_bass_kernel_spmd(nc, in_maps, core_ids=list(range(N_CORES)))

    scaled = np.concatenate([np.asarray(r["scaled"]) for r in res.results], axis=0)
    n_tiles = n_shard // P
    logz_rows = np.concatenate(
        [np.asarray(r["logz"]).reshape(P, n_tiles).T.ravel() for r in res.results]
    )
    picked = np.take_along_axis(
        scaled, labels.astype(np.int64).reshape(-1, 1), axis=1
    )[:, 0]
    loss = np.float32((logz_rows.astype(np.float64) - picked.astype(np.float64)).mean())
    return scaled, loss


# revision 24
# speedup vs baseline: 1.2566x; 1.2566x over previous
"""Adaptive temperature scaling kernel for Trainium2, 8 NeuronCores.

Data-parallel: rows sharded across 8 cores. Each core computes, per row x:
  LTS   = x . w_L
  s1    = sum(exp(x))            (inputs are ~N(0,1): no max-shift needed)
  t2    = sum(x * exp(x))
  H_hat = t2/s1 - ln(s1)         ( = sum(p*logp) )
  a     = LTS + w_H*H_hat/ln(C) + b
  T     = max(softplus(a), eps)  (stable: max(a,0) + ln(1+exp(-|a|)))
  scaled= x / T                  (wide output)
  logZ2 = max(x)/T + ln(sum(exp(x/T - max(x)/T)))
Host assembles: loss = mean(logZ2 - scaled[i, label_i]).
"""

import os
import sys

import numpy as np

sys.path.insert(0, "/opt/trn_rl_repo")

import concourse.bass as bass
import concourse.tile as tile
from concourse import mybir
from concourse.bass_utils import run_bass_kernel_spmd

F32 = mybir.dt.float32
AX = mybir.AxisListType
OP = mybir.AluOpType
AF = mybir.ActivationFunctionType


def split_multi_waits(nc, max_waits: int = 1):
    """This walrus build rejects instructions carrying more than one sem-wait
    (TRN2 ISA has a single sync-wait slot per instruction). Tile can attach
    several. Hoist extras onto no-op Drain instructions inserted immediately
    before the instruction in its block (same engine => same sequencer order,
    so semantics are identical)."""
    n_split = 0
    for fn in nc.m.functions:
        for blk in fn.blocks:
            insts = blk.instructions
            if not any(
                i.sync_info and i.sync_info.on_wait and len(i.sync_info.on_wait) > max_waits
                for i in insts
            ):
                continue
            new_list = []
            for inst in insts:
                si = inst.sync_info
                if si and si.on_wait and len(si.on_wait) > max_waits:
                    waits = list(si.on_wait)
                    for j, w in enumerate(waits[: -max_waits]):
                        carrier = mybir.InstDrain(
                            name=f"{inst.name}-wsplit{j}", ins=[], outs=[]
                        )
                        carrier.engine = inst.engine
                        carrier.sync_info = mybir.SyncInfo(on_wait=[w], on_update=[])
                        new_list.append(carrier)
                        n_split += 1
                    si.on_wait = waits[-max_waits:]
                new_list.append(inst)
            blk.instructions = new_list
    return n_split

N_CORES = 8
C = 128
P = 128
N_FULL = 524288
N_SHARD = N_FULL // N_CORES  # 65536

EPS = float(np.finfo(np.float32).eps)
INV_LNC = float(1.0 / np.log(np.float32(C)))


def build_nc(n_rows: int):
    """Build single-core Bass graph for an [n_rows, C] shard."""
    assert n_rows % P == 0
    n_tiles = n_rows // P

    # chunking: K tiles per chunk
    K = min(64, n_tiles)
    assert n_tiles % K == 0
    n_chunks = n_tiles // K

    nc = bass.Bass()

    x_ext = nc.declare_dram_parameter("x", [n_rows, C], F32, isOutput=False)
    wl_ext = nc.declare_dram_parameter("w_L", [1, C], F32, isOutput=False)
    wh_ext = nc.declare_dram_parameter("w_H", [1, 1], F32, isOutput=False)
    b_ext = nc.declare_dram_parameter("b", [1, 1], F32, isOutput=False)
    scaled_ext = nc.declare_dram_parameter("scaled", [n_rows, C], F32, isOutput=True)
    logz_ext = nc.declare_dram_parameter("logz", [n_rows], F32, isOutput=True)

    # DRAM views: row (t*128 + p) -> [p, t, c]
    x_v = x_ext.ap().rearrange("(t p) c -> p t c", p=P)
    sc_v = scaled_ext.ap().rearrange("(t p) c -> p t c", p=P)
    lz_v = logz_ext.ap().rearrange("(p t) -> p t", p=P)

    from contextlib import ExitStack

    with tile.TileContext(nc) as tc, ExitStack() as ctx:
        singles = ctx.enter_context(tc.tile_pool(name="singles", bufs=1))
        xpool = ctx.enter_context(tc.tile_pool(name="xpool", bufs=2))
        opool = ctx.enter_context(tc.tile_pool(name="opool", bufs=2))
        scratch = ctx.enter_context(tc.tile_pool(name="scratch", bufs=6))
        stats = ctx.enter_context(tc.tile_pool(name="stats", bufs=2))

        # constants
        wl_sb = singles.tile([P, C], F32)
        nc.sync.dma_start(out=wl_sb, in_=wl_ext.ap().to_broadcast([P, C]))
        wh_sb = singles.tile([P, 1], F32)
        nc.sync.dma_start(out=wh_sb, in_=wh_ext.ap().to_broadcast([P, 1]))
        b_sb = singles.tile([P, 1], F32)
        nc.sync.dma_start(out=b_sb, in_=b_ext.ap().to_broadcast([P, 1]))

        logz_all = singles.tile([P, n_tiles], F32)

        for kc in range(n_chunks):
            t0 = kc * K
            x_chunk = xpool.tile([P, K, C], F32)
            nc.sync.dma_start(out=x_chunk, in_=x_v[:, t0 : t0 + K, :])

            max1 = stats.tile([P, K], F32)
            s1 = stats.tile([P, K], F32)
            t2 = stats.tile([P, K], F32)
            lts = stats.tile([P, K], F32)

            # ---- phase A: raw stats per tile ----
            for t in range(K):
                xt = x_chunk[:, t, :]
                nc.vector.reduce_max(out=max1[:, t : t + 1], in_=xt, axis=AX.X)
                e = scratch.tile([P, C], F32, tag="e")
                nc.scalar.activation(
                    out=e, in_=xt, func=AF.Exp, accum_out=s1[:, t : t + 1]
                )
                j1 = scratch.tile([P, C], F32, tag="j1")
                nc.vector.scalar_tensor_tensor(
                    out=j1, in0=e, scalar=1.0, in1=xt,
                    op0=OP.mult, op1=OP.mult, accum_out=t2[:, t : t + 1],
                )
                j2 = scratch.tile([P, C], F32, tag="j2")
                nc.vector.scalar_tensor_tensor(
                    out=j2, in0=xt, scalar=1.0, in1=wl_sb,
                    op0=OP.mult, op1=OP.mult, accum_out=lts[:, t : t + 1],
                )

            # ---- phase B: chunk epilogue on [P, K] stats ----
            lns1 = stats.tile([P, K], F32)
            nc.scalar.activation(out=lns1, in_=s1, func=AF.Ln)
            r1 = stats.tile([P, K], F32)
            nc.vector.reciprocal(out=r1, in_=s1)
            hh = stats.tile([P, K], F32)
            nc.vector.tensor_tensor(out=hh, in0=t2, in1=r1, op=OP.mult)
            nc.vector.tensor_tensor(out=hh, in0=hh, in1=lns1, op=OP.subtract)
            # a = LTS + w_H*hh/lnC + b
            a = stats.tile([P, K], F32)
            nc.vector.tensor_scalar(
                out=a, in0=hh, scalar1=wh_sb, scalar2=INV_LNC,
                op0=OP.mult, op1=OP.mult,
            )
            nc.vector.tensor_tensor(out=a, in0=a, in1=lts, op=OP.add)
            nc.vector.tensor_scalar(out=a, in0=a, scalar1=b_sb, scalar2=None, op0=OP.add)
            # T = max(softplus(a), eps); softplus = max(a,0) + ln(1+exp(-|a|))
            absa = stats.tile([P, K], F32)
            nc.scalar.activation(out=absa, in_=a, func=AF.Abs)
            en = stats.tile([P, K], F32)
            nc.scalar.activation(out=en, in_=absa, func=AF.Exp, scale=-1.0)
            # log1p(en) with full relative precision for tiny en:
            #   en >= 3e-2: Ln(1 + en)  (f32 rounding of 1+en is harmless)
            #   en <  3e-2: en*(1 - en/2 + en^2/3)  (|err| <= en^4/4)
            lnb = stats.tile([P, K], F32)
            nc.scalar.activation(out=lnb, in_=en, func=AF.Ln, bias=1.0)
            q = stats.tile([P, K], F32)
            nc.vector.tensor_scalar(
                out=q, in0=en, scalar1=-1.0 / 3.0, scalar2=0.5, op0=OP.mult, op1=OP.add
            )
            nc.vector.tensor_tensor(out=q, in0=en, in1=q, op=OP.mult)
            nc.vector.tensor_scalar(
                out=q, in0=q, scalar1=-1.0, scalar2=1.0, op0=OP.mult, op1=OP.add
            )
            poly = stats.tile([P, K], F32)
            nc.vector.tensor_tensor(out=poly, in0=en, in1=q, op=OP.mult)
            msk = stats.tile([P, K], F32)
            nc.vector.tensor_scalar(
                out=msk, in0=en, scalar1=3.0e-2, scalar2=None, op0=OP.is_lt
            )
            l1p = stats.tile([P, K], F32)
            nc.vector.tensor_tensor(out=poly, in0=poly, in1=msk, op=OP.mult)
            nc.vector.tensor_scalar(
                out=msk, in0=msk, scalar1=-1.0, scalar2=1.0, op0=OP.mult, op1=OP.add
            )
            nc.vector.tensor_tensor(out=l1p, in0=lnb, in1=msk, op=OP.mult)
            nc.vector.tensor_tensor(out=l1p, in0=l1p, in1=poly, op=OP.add)
            tt = stats.tile([P, K], F32)
            nc.vector.tensor_scalar(out=tt, in0=a, scalar1=0.0, scalar2=None, op0=OP.max)
            nc.vector.tensor_tensor(out=tt, in0=tt, in1=l1p, op=OP.add)
            nc.vector.tensor_scalar(out=tt, in0=tt, scalar1=EPS, scalar2=None, op0=OP.max)
            invt = stats.tile([P, K], F32)
            nc.vector.reciprocal(out=invt, in_=tt)
            max2 = stats.tile([P, K], F32)
            nc.vector.tensor_tensor(out=max2, in0=max1, in1=invt, op=OP.mult)
            nmax2 = stats.tile([P, K], F32)
            nc.vector.tensor_scalar(
                out=nmax2, in0=max2, scalar1=-1.0, scalar2=None, op0=OP.mult
            )

            # ---- phase C: scaled output + s2 ----
            s2 = stats.tile([P, K], F32)
            sc_chunk = opool.tile([P, K, C], F32)
            for t in range(K):
                xt = x_chunk[:, t, :]
                st = sc_chunk[:, t, :]
                nc.vector.tensor_scalar(
                    out=st, in0=xt, scalar1=invt[:, t : t + 1], scalar2=None,
                    op0=OP.mult,
                )
                e2 = scratch.tile([P, C], F32, tag="e2")
                nc.scalar.activation(
                    out=e2, in_=st, func=AF.Exp, bias=nmax2[:, t : t + 1],
                    accum_out=s2[:, t : t + 1],
                )
            nc.sync.dma_start(out=sc_v[:, t0 : t0 + K, :], in_=sc_chunk)

            # ---- phase D: logZ2 ----
            lns2 = stats.tile([P, K], F32)
            nc.scalar.activation(out=lns2, in_=s2, func=AF.Ln)
            nc.vector.tensor_tensor(
                out=logz_all[:, t0 : t0 + K], in0=max2, in1=lns2, op=OP.add
            )

        nc.sync.dma_start(out=lz_v, in_=logz_all)

    return nc


def build_nc_v1(
    n_rows: int,
    p2_engine: str = "gpsimd",
    copy_split: float = 0.5,
    e2_mode: str = "acc",
    scaled_engine: str = "vector",
    sh2_engine: str = "vector",
):
    """v1: PE-transpose front-end. Per-row reductions (s1, t2, LTS) become
    float32r matmuls with the class-major tile as weights and a ones/w_L
    column as rhs — stats land directly in [row-partition, tile] layout in
    PSUM. DVE keeps only max1 + the temperature application; ACT does the two
    exp passes; GPSIMD does the one elementwise multiply (E*X)."""
    from contextlib import ExitStack

    from concourse.masks import make_identity

    F32R = mybir.dt.float32r
    assert n_rows % P == 0
    n_tiles = n_rows // P
    K = min(16, n_tiles)  # tiles per chunk
    assert n_tiles % K == 0
    n_chunks = n_tiles // K
    G = min(4, K)  # tiles per PSUM transpose group (one 2KB bank)
    assert K % G == 0

    nc = bass.Bass()

    x_ext = nc.declare_dram_parameter("x", [n_rows, C], F32, isOutput=False)
    wl_ext = nc.declare_dram_parameter("w_L", [1, C], F32, isOutput=False)
    wh_ext = nc.declare_dram_parameter("w_H", [1, 1], F32, isOutput=False)
    b_ext = nc.declare_dram_parameter("b", [1, 1], F32, isOutput=False)
    scaled_ext = nc.declare_dram_parameter("scaled", [n_rows, C], F32, isOutput=True)
    logz_ext = nc.declare_dram_parameter("logz", [n_rows], F32, isOutput=True)

    x_v = x_ext.ap().rearrange("(t p) c -> p t c", p=P)
    sc_v = scaled_ext.ap().rearrange("(t p) c -> p t c", p=P)
    lz_v = logz_ext.ap().rearrange("(p t) -> p t", p=P)

    with tile.TileContext(nc) as tc, ExitStack() as ctx:
        singles = ctx.enter_context(tc.tile_pool(name="singles", bufs=1))
        xpool = ctx.enter_context(tc.tile_pool(name="xpool", bufs=2))
        opool = ctx.enter_context(tc.tile_pool(name="opool", bufs=2))
        cmpool = ctx.enter_context(tc.tile_pool(name="cmpool", bufs=2))
        stats = ctx.enter_context(tc.tile_pool(name="stats", bufs=2))
        psum_t = ctx.enter_context(tc.tile_pool(name="psum_t", bufs=3, space="PSUM"))
        psum_s = ctx.enter_context(tc.tile_pool(name="psum_s", bufs=2, space="PSUM"))
        psum_j = ctx.enter_context(tc.tile_pool(name="psum_j", bufs=2, space="PSUM"))

        # constants
        ident = singles.tile([P, P], F32)
        make_identity(nc, ident)
        onesf = singles.tile([P, 2], F32)
        nc.vector.memset(onesf, 1.0)
        ones_r = singles.tile([P, 2], F32R)
        nc.vector.tensor_copy(ones_r, onesf)
        # [w_L | ones] column pair (PSUM matmul outputs must be >=2 wide)
        wlcolf = singles.tile([P, 2], F32)
        nc.sync.dma_start(out=wlcolf[:, 0:1], in_=wl_ext.ap().rearrange("a c -> c a"))
        nc.vector.memset(wlcolf[:, 1:2], 1.0)
        wl_r = singles.tile([P, 2], F32R)
        nc.vector.tensor_copy(wl_r, wlcolf)
        wh_sb = singles.tile([P, 1], F32)
        nc.sync.dma_start(out=wh_sb, in_=wh_ext.ap().to_broadcast([P, 1]))
        b_sb = singles.tile([P, 1], F32)
        nc.sync.dma_start(out=b_sb, in_=b_ext.ap().to_broadcast([P, 1]))

        logz_all = singles.tile([P, n_tiles], F32)

        for kc in range(n_chunks):
            t0 = kc * K
            x_chunk = xpool.tile([P, K, C], F32)
            nc.sync.dma_start(out=x_chunk, in_=x_v[:, t0 : t0 + K, :])

            max1 = stats.tile([P, K], F32)
            nc.vector.reduce_max(out=max1, in_=x_chunk, axis=AX.X)

            et_chunk = cmpool.tile([P, K, C], F32R, tag="et")
            xt_chunk = cmpool.tile([P, K, C], F32R, tag="xt")
            for g in range(K // G):
                ps = psum_t.tile([P, G, P], F32)
                for j in range(G):
                    nc.tensor.transpose(ps[:, j, :], x_chunk[:, g * G + j, :], ident)
                sl = slice(g * G, (g + 1) * G)
                nc.scalar.activation(out=et_chunk[:, sl, :], in_=ps, func=AF.Exp)
                # split the f32r copy between ACT and DVE for balance
                if g < int((K // G) * copy_split):
                    nc.vector.tensor_copy(xt_chunk[:, sl, :], ps)
                else:
                    nc.scalar.activation(
                        out=xt_chunk[:, sl, :], in_=ps, func=AF.Identity
                    )

            p2_chunk = cmpool.tile([P, K, C], F32R, tag="p2")
            eng = nc.gpsimd if p2_engine == "gpsimd" else nc.vector
            eng.tensor_tensor(out=p2_chunk, in0=et_chunk, in1=xt_chunk, op=OP.mult)

            # stats matmuls: lhsT = class-major tile, rhs = 2-wide columns
            # (PSUM mm outputs must be >=2 elems); layout per tile: 6 cols
            # [s1, s1, t2, t2, LTS, rowsum]
            st_ps = psum_s.tile([P, 6 * K], F32)
            for t in range(K):
                nc.tensor.matmul(
                    st_ps[:, 6 * t : 6 * t + 2], et_chunk[:, t, :], ones_r,
                    start=True, stop=True,
                )
                nc.tensor.matmul(
                    st_ps[:, 6 * t + 2 : 6 * t + 4], p2_chunk[:, t, :], ones_r,
                    start=True, stop=True,
                )
                nc.tensor.matmul(
                    st_ps[:, 6 * t + 4 : 6 * t + 6], xt_chunk[:, t, :], wl_r,
                    start=True, stop=True,
                )
            st_sb = stats.tile([P, 6 * K], F32)
            nc.vector.tensor_copy(st_sb, st_ps)
            st6 = st_sb.rearrange("p (k six) -> p k six", six=6)
            s1 = st6[:, :, 0]
            t2 = st6[:, :, 2]
            lts = st6[:, :, 4]

            # ---- epilogue on [P, K] stats ----
            lns1 = stats.tile([P, K], F32)
            nc.scalar.activation(out=lns1, in_=s1, func=AF.Ln)
            r1 = stats.tile([P, K], F32)
            nc.vector.reciprocal(out=r1, in_=s1)
            hh = stats.tile([P, K], F32)
            nc.vector.tensor_tensor(out=hh, in0=t2, in1=r1, op=OP.mult)
            nc.vector.tensor_tensor(out=hh, in0=hh, in1=lns1, op=OP.subtract)
            a = stats.tile([P, K], F32)
            nc.vector.tensor_scalar(
                out=a, in0=hh, scalar1=wh_sb, scalar2=INV_LNC,
                op0=OP.mult, op1=OP.mult,
            )
            nc.vector.tensor_tensor(out=a, in0=a, in1=lts, op=OP.add)
            nc.vector.tensor_scalar(
                out=a, in0=a, scalar1=b_sb, scalar2=None, op0=OP.add
            )
            absa = stats.tile([P, K], F32)
            nc.scalar.activation(out=absa, in_=a, func=AF.Abs)
            en = stats.tile([P, K], F32)
            nc.scalar.activation(out=en, in_=absa, func=AF.Exp, scale=-1.0)
            lnb = stats.tile([P, K], F32)
            nc.scalar.activation(out=lnb, in_=en, func=AF.Ln, bias=1.0)
            q = stats.tile([P, K], F32)
            nc.vector.tensor_scalar(
                out=q, in0=en, scalar1=-1.0 / 3.0, scalar2=0.5,
                op0=OP.mult, op1=OP.add,
            )
            nc.vector.tensor_tensor(out=q, in0=en, in1=q, op=OP.mult)
            nc.vector.tensor_scalar(
                out=q, in0=q, scalar1=-1.0, scalar2=1.0, op0=OP.mult, op1=OP.add
            )
            poly = stats.tile([P, K], F32)
            nc.vector.tensor_tensor(out=poly, in0=en, in1=q, op=OP.mult)
            msk = stats.tile([P, K], F32)
            nc.vector.tensor_scalar(
                out=msk, in0=en, scalar1=3.0e-2, scalar2=None, op0=OP.is_lt
            )
            l1p = stats.tile([P, K], F32)
            nc.vector.tensor_tensor(out=poly, in0=poly, in1=msk, op=OP.mult)
            nc.vector.tensor_scalar(
                out=msk, in0=msk, scalar1=-1.0, scalar2=1.0, op0=OP.mult, op1=OP.add
            )
            nc.vector.tensor_tensor(out=l1p, in0=lnb, in1=msk, op=OP.mult)
            nc.vector.tensor_tensor(out=l1p, in0=l1p, in1=poly, op=OP.add)
            tt = stats.tile([P, K], F32)
            nc.vector.tensor_scalar(
                out=tt, in0=a, scalar1=0.0, scalar2=None, op0=OP.max
            )
            nc.vector.tensor_tensor(out=tt, in0=tt, in1=l1p, op=OP.add)
            nc.vector.tensor_scalar(
                out=tt, in0=tt, scalar1=EPS, scalar2=None, op0=OP.max
            )
            invt = stats.tile([P, K], F32)
            nc.vector.reciprocal(out=invt, in_=tt)
            max2 = stats.tile([P, K], F32)
            nc.vector.tensor_tensor(out=max2, in0=max1, in1=invt, op=OP.mult)
            nmax2 = stats.tile([P, K], F32)
            nc.vector.tensor_scalar(
                out=nmax2, in0=max2, scalar1=-1.0, scalar2=None, op0=OP.mult
            )

            # ---- scaled output + s2 ----
            s2 = stats.tile([P, K], F32)
            sc_chunk = opool.tile([P, K, C], F32)
            if e2_mode == "acc":
                for t in range(K):
                    xt_ = x_chunk[:, t, :]
                    st_ = sc_chunk[:, t, :]
                    nc.vector.tensor_scalar(
                        out=st_, in0=xt_, scalar1=invt[:, t : t + 1], scalar2=None,
                        op0=OP.mult,
                    )
                    e2_ps = psum_j.tile([P, P], F32, tag="e2")
                    nc.scalar.activation(
                        out=e2_ps, in_=st_, func=AF.Exp, bias=nmax2[:, t : t + 1],
                        accum_out=s2[:, t : t + 1],
                    )
            else:
                # chunk-wide: scaled = X*invT (stride-0 bcast), sh2 = scaled-max2,
                # E2 = exp(sh2), s2 = segmented reduce
                eng_sc = nc.vector if scaled_engine == "vector" else nc.gpsimd
                eng_sh = nc.vector if sh2_engine == "vector" else nc.gpsimd
                eng_sc.scalar_tensor_tensor(
                    out=sc_chunk, in0=x_chunk, scalar=1.0,
                    in1=invt.broadcast_to([P, K, C]),
                    op0=OP.mult, op1=OP.mult,
                )
                sh2 = cmpool.tile([P, K, C], F32, tag="p2")
                eng_sh.scalar_tensor_tensor(
                    out=sh2, in0=sc_chunk, scalar=1.0,
                    in1=nmax2.broadcast_to([P, K, C]),
                    op0=OP.mult, op1=OP.add,
                )
                e2 = cmpool.tile([P, K, C], F32, tag="e2w")
                nc.scalar.activation(out=e2, in_=sh2, func=AF.Exp)
                nc.vector.tensor_reduce(out=s2, in_=e2, axis=AX.X, op=OP.add)
            nc.sync.dma_start(out=sc_v[:, t0 : t0 + K, :], in_=sc_chunk)

            lns2 = stats.tile([P, K], F32)
            nc.scalar.activation(out=lns2, in_=s2, func=AF.Ln)
            nc.vector.tensor_tensor(
                out=logz_all[:, t0 : t0 + K], in0=max2, in1=lns2, op=OP.add
            )

        nc.sync.dma_start(out=lz_v, in_=logz_all)

    return nc


def build_nc_v3(
    n_rows: int,
    K: int = 16,
    SC: int = 4,
    sh2_engine: str = "gpsimd",
    scaled_engine: str = "vector",
    wl_is_ones: bool = True,
):
    """v3: like v2 but
    - E and the t2 product run in bf16 (DVE tensor_tensor gets its 2x mode;
      the later f32 reductions are unchanged, errors ~1e-3 relative on
      s1/t2 which is far inside the 2e-2 gate)
    - the scalar epilogue is batched over SC chunks (amortizes the ~400-cycle
      per-instruction SBUF bubble that cost 145us in v2)
    - separate pool tags per wide intermediate so chunks pipeline deeply
    """
    from contextlib import ExitStack

    BF16 = mybir.dt.bfloat16
    assert n_rows % P == 0
    n_tiles = n_rows // P
    K = min(K, n_tiles)
    assert n_tiles % K == 0
    n_chunks = n_tiles // K
    SC = min(SC, n_chunks)
    assert n_chunks % SC == 0
    KS = K * SC  # tiles per superchunk

    nc = bass.Bass()

    x_ext = nc.declare_dram_parameter("x", [n_rows, C], F32, isOutput=False)
    wl_ext = nc.declare_dram_parameter("w_L", [1, C], F32, isOutput=False)
    wh_ext = nc.declare_dram_parameter("w_H", [1, 1], F32, isOutput=False)
    b_ext = nc.declare_dram_parameter("b", [1, 1], F32, isOutput=False)
    scaled_ext = nc.declare_dram_parameter("scaled", [n_rows, C], F32, isOutput=True)
    logz_ext = nc.declare_dram_parameter("logz", [n_rows], F32, isOutput=True)

    x_v = x_ext.ap().rearrange("(t p) c -> p t c", p=P)
    sc_v = scaled_ext.ap().rearrange("(t p) c -> p t c", p=P)
    lz_v = logz_ext.ap().rearrange("(p t) -> p t", p=P)

    def E(name):
        return {"gpsimd": nc.gpsimd, "vector": nc.vector}[name]

    with tile.TileContext(nc) as tc, ExitStack() as ctx:
        singles = ctx.enter_context(tc.tile_pool(name="singles", bufs=1))
        xpool = ctx.enter_context(
            tc.tile_pool(name="xpool", bufs=(SC + 2 if K <= 16 else SC + 1))
        )
        opool = ctx.enter_context(
            tc.tile_pool(name="opool", bufs=(3 if K <= 16 else 2))
        )
        wide = ctx.enter_context(tc.tile_pool(name="wide", bufs=2))
        stats = ctx.enter_context(tc.tile_pool(name="stats", bufs=2))

        wl_sb = singles.tile([P, C], F32)
        nc.sync.dma_start(out=wl_sb, in_=wl_ext.ap().to_broadcast([P, C]))
        wh_sb = singles.tile([P, 1], F32)
        nc.sync.dma_start(out=wh_sb, in_=wh_ext.ap().to_broadcast([P, 1]))
        b_sb = singles.tile([P, 1], F32)
        nc.sync.dma_start(out=b_sb, in_=b_ext.ap().to_broadcast([P, 1]))

        logz_all = singles.tile([P, n_tiles], F32)

        for sck in range(n_chunks // SC):
            st0 = sck * KS
            x_chunks = []
            max1 = stats.tile([P, KS], F32, tag="max1")
            s1 = stats.tile([P, KS], F32, tag="s1")
            t2 = stats.tile([P, KS], F32, tag="t2")
            lts = stats.tile([P, KS], F32, tag="lts")
            # ---- phase A per chunk ----
            for j in range(SC):
                t0 = st0 + j * K
                ks = slice(j * K, (j + 1) * K)
                x_chunk = xpool.tile([P, K, C], F32)
                x_chunks.append(x_chunk)
                nc.sync.dma_start(out=x_chunk, in_=x_v[:, t0 : t0 + K, :])

                nc.vector.reduce_max(out=max1[:, ks], in_=x_chunk, axis=AX.X)
                eb = wide.tile([P, K, C], BF16, tag="eb")
                nc.scalar.activation(out=eb, in_=x_chunk, func=AF.Exp)
                nc.vector.tensor_reduce(out=s1[:, ks], in_=eb, axis=AX.X, op=OP.add)
                xb = wide.tile([P, K, C], BF16, tag="xb")
                nc.vector.tensor_copy(xb, x_chunk)
                p2 = wide.tile([P, K, C], BF16, tag="p2")
                nc.vector.tensor_tensor(out=p2, in0=eb, in1=xb, op=OP.mult)
                nc.vector.tensor_reduce(out=t2[:, ks], in_=p2, axis=AX.X, op=OP.add)
                if wl_is_ones:
                    nc.vector.tensor_reduce(
                        out=lts[:, ks], in_=x_chunk, axis=AX.X, op=OP.add
                    )
                else:
                    lw = wide.tile([P, K, C], F32, tag="lw")
                    nc.vector.tensor_tensor(
                        out=lw, in0=x_chunk,
                        in1=wl_sb.rearrange("p (k c) -> p k c", k=1).broadcast_to(
                            [P, K, C]
                        ),
                        op=OP.mult,
                    )
                    nc.vector.tensor_reduce(
                        out=lts[:, ks], in_=lw, axis=AX.X, op=OP.add
                    )

            # ---- epilogue batched over the superchunk [P, KS] ----
            lns1 = stats.tile([P, KS], F32, tag="lns1")
            nc.scalar.activation(out=lns1, in_=s1, func=AF.Ln)
            r1 = stats.tile([P, KS], F32, tag="r1")
            nc.vector.reciprocal(out=r1, in_=s1)
            hh = stats.tile([P, KS], F32, tag="hh")
            nc.vector.tensor_tensor(out=hh, in0=t2, in1=r1, op=OP.mult)
            nc.vector.tensor_tensor(out=hh, in0=hh, in1=lns1, op=OP.subtract)
            a = stats.tile([P, KS], F32, tag="a")
            nc.vector.tensor_scalar(
                out=a, in0=hh, scalar1=wh_sb, scalar2=INV_LNC,
                op0=OP.mult, op1=OP.mult,
            )
            nc.vector.tensor_tensor(out=a, in0=a, in1=lts, op=OP.add)
            nc.vector.tensor_scalar(
                out=a, in0=a, scalar1=b_sb, scalar2=None, op0=OP.add
            )
            absa = stats.tile([P, KS], F32, tag="absa")
            nc.scalar.activation(out=absa, in_=a, func=AF.Abs)
            en = stats.tile([P, KS], F32, tag="en")
            nc.scalar.activation(out=en, in_=absa, func=AF.Exp, scale=-1.0)
            lnb = stats.tile([P, KS], F32, tag="lnb")
            nc.scalar.activation(out=lnb, in_=en, func=AF.Ln, bias=1.0)
            q = stats.tile([P, KS], F32, tag="q")
            nc.vector.tensor_scalar(
                out=q, in0=en, scalar1=-1.0 / 3.0, scalar2=0.5,
                op0=OP.mult, op1=OP.add,
            )
            nc.vector.tensor_tensor(out=q, in0=en, in1=q, op=OP.mult)
            nc.vector.tensor_scalar(
                out=q, in0=q, scalar1=-1.0, scalar2=1.0, op0=OP.mult, op1=OP.add
            )
            poly = stats.tile([P, KS], F32, tag="poly")
            nc.vector.tensor_tensor(out=poly, in0=en, in1=q, op=OP.mult)
            msk = stats.tile([P, KS], F32, tag="msk")
            nc.vector.tensor_scalar(
                out=msk, in0=en, scalar1=3.0e-2, scalar2=None, op0=OP.is_lt
            )
            l1p = stats.tile([P, KS], F32, tag="l1p")
            nc.vector.tensor_tensor(out=poly, in0=poly, in1=msk, op=OP.mult)
            nc.vector.tensor_scalar(
                out=msk, in0=msk, scalar1=-1.0, scalar2=1.0, op0=OP.mult, op1=OP.add
            )
            nc.vector.tensor_tensor(out=l1p, in0=lnb, in1=msk, op=OP.mult)
            nc.vector.tensor_tensor(out=l1p, in0=l1p, in1=poly, op=OP.add)
            tt = stats.tile([P, KS], F32, tag="tt")
            nc.vector.tensor_scalar(
                out=tt, in0=a, scalar1=0.0, scalar2=None, op0=OP.max
            )
            nc.vector.tensor_tensor(out=tt, in0=tt, in1=l1p, op=OP.add)
            nc.vector.tensor_scalar(
                out=tt, in0=tt, scalar1=EPS, scalar2=None, op0=OP.max
            )
            invt = stats.tile([P, KS], F32, tag="invt")
            nc.vector.reciprocal(out=invt, in_=tt)
            max2 = stats.tile([P, KS], F32, tag="max2")
            nc.vector.tensor_tensor(out=max2, in0=max1, in1=invt, op=OP.mult)
            nmax2 = stats.tile([P, KS], F32, tag="nmax2")
            nc.vector.tensor_scalar(
                out=nmax2, in0=max2, scalar1=-1.0, scalar2=None, op0=OP.mult
            )

            # ---- phase C per chunk ----
            s2 = stats.tile([P, KS], F32, tag="s2")
            for j in range(SC):
                t0 = st0 + j * K
                ks = slice(j * K, (j + 1) * K)
                x_chunk = x_chunks[j]
                sc_chunk = opool.tile([P, K, C], F32)
                E(scaled_engine).tensor_tensor(
                    out=sc_chunk, in0=x_chunk,
                    in1=invt[:, ks].broadcast_to([P, K, C]), op=OP.mult,
                )
                sh2 = wide.tile([P, K, C], F32, tag="sh2")
                E(sh2_engine).tensor_tensor(
                    out=sh2, in0=sc_chunk,
                    in1=nmax2[:, ks].broadcast_to([P, K, C]), op=OP.add,
                )
                e2 = wide.tile([P, K, C], BF16, tag="e2")
                nc.scalar.activation(out=e2, in_=sh2, func=AF.Exp)
                nc.vector.tensor_reduce(out=s2[:, ks], in_=e2, axis=AX.X, op=OP.add)
                nc.sync.dma_start(out=sc_v[:, t0 : t0 + K, :], in_=sc_chunk)

            lns2 = stats.tile([P, KS], F32, tag="lns2")
            nc.scalar.activation(out=lns2, in_=s2, func=AF.Ln)
            nc.vector.tensor_tensor(
                out=logz_all[:, st0 : st0 + KS], in0=max2, in1=lns2, op=OP.add
            )

        nc.sync.dma_start(out=lz_v, in_=logz_all)

    return nc


def build_nc_v2(
    n_rows: int,
    K: int = 32,
    p2_engine: str = "gpsimd",
    sh2_engine: str = "gpsimd",
    scaled_engine: str = "vector",
    wl_is_ones: bool = True,
):
    """v2: row-major, chunk-wide ops only (no PE, no per-tile instructions).
    Per chunk of K 128-row tiles:
      E   = exp(X)                (ACT, one big-FD instr)
      max1, s1=red(E), t2=red(X*E), rowsum=red(X) [or red(X*w_L)], per-row
      stats via segmented DVE reduces; X*E on GPSIMD.
      epilogue -> invT, -max2 (batched [P,K])
      scaled = X*invT  (stt with stride-0 broadcast of invT)
      sh2 = scaled - max2 (stt broadcast), E2 = exp(sh2), s2 = red(E2)
      logZ2 = max2 + ln(s2)"""
    from contextlib import ExitStack

    assert n_rows % P == 0
    n_tiles = n_rows // P
    K = min(K, n_tiles)
    assert n_tiles % K == 0
    n_chunks = n_tiles // K

    nc = bass.Bass()

    x_ext = nc.declare_dram_parameter("x", [n_rows, C], F32, isOutput=False)
    wl_ext = nc.declare_dram_parameter("w_L", [1, C], F32, isOutput=False)
    wh_ext = nc.declare_dram_parameter("w_H", [1, 1], F32, isOutput=False)
    b_ext = nc.declare_dram_parameter("b", [1, 1], F32, isOutput=False)
    scaled_ext = nc.declare_dram_parameter("scaled", [n_rows, C], F32, isOutput=True)
    logz_ext = nc.declare_dram_parameter("logz", [n_rows], F32, isOutput=True)

    x_v = x_ext.ap().rearrange("(t p) c -> p t c", p=P)
    sc_v = scaled_ext.ap().rearrange("(t p) c -> p t c", p=P)
    lz_v = logz_ext.ap().rearrange("(p t) -> p t", p=P)

    def E(name):
        return {"gpsimd": nc.gpsimd, "vector": nc.vector}[name]

    with tile.TileContext(nc) as tc, ExitStack() as ctx:
        singles = ctx.enter_context(tc.tile_pool(name="singles", bufs=1))
        xpool = ctx.enter_context(tc.tile_pool(name="xpool", bufs=2))
        opool = ctx.enter_context(tc.tile_pool(name="opool", bufs=2))
        wide = ctx.enter_context(tc.tile_pool(name="wide", bufs=2))
        stats = ctx.enter_context(tc.tile_pool(name="stats", bufs=2))

        wl_sb = singles.tile([P, C], F32)
        nc.sync.dma_start(out=wl_sb, in_=wl_ext.ap().to_broadcast([P, C]))
        wh_sb = singles.tile([P, 1], F32)
        nc.sync.dma_start(out=wh_sb, in_=wh_ext.ap().to_broadcast([P, 1]))
        b_sb = singles.tile([P, 1], F32)
        nc.sync.dma_start(out=b_sb, in_=b_ext.ap().to_broadcast([P, 1]))

        logz_all = singles.tile([P, n_tiles], F32)

        for kc in range(n_chunks):
            t0 = kc * K
            x_chunk = xpool.tile([P, K, C], F32)
            nc.sync.dma_start(out=x_chunk, in_=x_v[:, t0 : t0 + K, :])

            max1 = stats.tile([P, K], F32)
            nc.vector.reduce_max(out=max1, in_=x_chunk, axis=AX.X)

            e_chunk = wide.tile([P, K, C], F32, tag="e")
            nc.scalar.activation(out=e_chunk, in_=x_chunk, func=AF.Exp)
            s1 = stats.tile([P, K], F32)
            nc.vector.tensor_reduce(out=s1, in_=e_chunk, axis=AX.X, op=OP.add)

            p2 = wide.tile([P, K, C], F32, tag="p2")
            E(p2_engine).tensor_tensor(out=p2, in0=e_chunk, in1=x_chunk, op=OP.mult)
            t2 = stats.tile([P, K], F32)
            nc.vector.tensor_reduce(out=t2, in_=p2, axis=AX.X, op=OP.add)

            lts = stats.tile([P, K], F32)
            if wl_is_ones:
                nc.vector.tensor_reduce(out=lts, in_=x_chunk, axis=AX.X, op=OP.add)
            else:
                lw = wide.tile([P, K, C], F32, tag="lw")
                nc.vector.tensor_tensor(
                    out=lw, in0=x_chunk,
                    in1=wl_sb.rearrange("p (k c) -> p k c", k=1).broadcast_to([P, K, C]),
                    op=OP.mult,
                )
                nc.vector.tensor_reduce(out=lts, in_=lw, axis=AX.X, op=OP.add)

            # ---- epilogue on [P, K] ----
            lns1 = stats.tile([P, K], F32)
            nc.scalar.activation(out=lns1, in_=s1, func=AF.Ln)
            r1 = stats.tile([P, K], F32)
            nc.vector.reciprocal(out=r1, in_=s1)
            hh = stats.tile([P, K], F32)
            nc.vector.tensor_tensor(out=hh, in0=t2, in1=r1, op=OP.mult)
            nc.vector.tensor_tensor(out=hh, in0=hh, in1=lns1, op=OP.subtract)
            a = stats.tile([P, K], F32)
            nc.vector.tensor_scalar(
                out=a, in0=hh, scalar1=wh_sb, scalar2=INV_LNC,
                op0=OP.mult, op1=OP.mult,
            )
            nc.vector.tensor_tensor(out=a, in0=a, in1=lts, op=OP.add)
            nc.vector.tensor_scalar(
                out=a, in0=a, scalar1=b_sb, scalar2=None, op0=OP.add
            )
            absa = stats.tile([P, K], F32)
            nc.scalar.activation(out=absa, in_=a, func=AF.Abs)
            en = stats.tile([P, K], F32)
            nc.scalar.activation(out=en, in_=absa, func=AF.Exp, scale=-1.0)
            lnb = stats.tile([P, K], F32)
            nc.scalar.activation(out=lnb, in_=en, func=AF.Ln, bias=1.0)
            q = stats.tile([P, K], F32)
            nc.vector.tensor_scalar(
                out=q, in0=en, scalar1=-1.0 / 3.0, scalar2=0.5,
                op0=OP.mult, op1=OP.add,
            )
            nc.vector.tensor_tensor(out=q, in0=en, in1=q, op=OP.mult)
            nc.vector.tensor_scalar(
                out=q, in0=q, scalar1=-1.0, scalar2=1.0, op0=OP.mult, op1=OP.add
            )
            poly = stats.tile([P, K], F32)
            nc.vector.tensor_tensor(out=poly, in0=en, in1=q, op=OP.mult)
            msk = stats.tile([P, K], F32)
            nc.vector.tensor_scalar(
                out=msk, in0=en, scalar1=3.0e-2, scalar2=None, op0=OP.is_lt
            )
            l1p = stats.tile([P, K], F32)
            nc.vector.tensor_tensor(out=poly, in0=poly, in1=msk, op=OP.mult)
            nc.vector.tensor_scalar(
                out=msk, in0=msk, scalar1=-1.0, scalar2=1.0, op0=OP.mult, op1=OP.add
            )
            nc.vector.tensor_tensor(out=l1p, in0=lnb, in1=msk, op=OP.mult)
            nc.vector.tensor_tensor(out=l1p, in0=l1p, in1=poly, op=OP.add)
            tt = stats.tile([P, K], F32)
            nc.vector.tensor_scalar(
                out=tt, in0=a, scalar1=0.0, scalar2=None, op0=OP.max
            )
            nc.vector.tensor_tensor(out=tt, in0=tt, in1=l1p, op=OP.add)
            nc.vector.tensor_scalar(
                out=tt, in0=tt, scalar1=EPS, scalar2=None, op0=OP.max
            )
            invt = stats.tile([P, K], F32)
            nc.vector.reciprocal(out=invt, in_=tt)
            max2 = stats.tile([P, K], F32)
            nc.vector.tensor_tensor(out=max2, in0=max1, in1=invt, op=OP.mult)
            nmax2 = stats.tile([P, K], F32)
            nc.vector.tensor_scalar(
                out=nmax2, in0=max2, scalar1=-1.0, scalar2=None, op0=OP.mult
            )

            # ---- phase C ----
            sc_chunk = opool.tile([P, K, C], F32)
            E(scaled_engine).tensor_tensor(
                out=sc_chunk, in0=x_chunk, in1=invt.broadcast_to([P, K, C]),
                op=OP.mult,
            )
            sh2 = wide.tile([P, K, C], F32, tag="e")
            E(sh2_engine).tensor_tensor(
                out=sh2, in0=sc_chunk, in1=nmax2.broadcast_to([P, K, C]),
                op=OP.add,
            )
            e2 = wide.tile([P, K, C], F32, tag="p2")
            nc.scalar.activation(out=e2, in_=sh2, func=AF.Exp)
            s2 = stats.tile([P, K], F32)
            nc.vector.tensor_reduce(out=s2, in_=e2, axis=AX.X, op=OP.add)
            nc.sync.dma_start(out=sc_v[:, t0 : t0 + K, :], in_=sc_chunk)

            lns2 = stats.tile([P, K], F32)
            nc.scalar.activation(out=lns2, in_=s2, func=AF.Ln)
            nc.vector.tensor_tensor(
                out=logz_all[:, t0 : t0 + K], in0=max2, in1=lns2, op=OP.add
            )

        nc.sync.dma_start(out=lz_v, in_=logz_all)

    return nc


_NC_CACHE: dict[tuple, object] = {}


def _get_nc(n_rows: int, wl_is_ones: bool = True):
    key = (n_rows, wl_is_ones)
    if key not in _NC_CACHE:
        nc = build_nc_v3(n_rows, wl_is_ones=wl_is_ones)
        split_multi_waits(nc)  # HW compiler path only; CoreSim rejects carriers
        _NC_CACHE[key] = nc
    return _NC_CACHE[key]


def kernel(Simple_vector, label_list, w_L, w_H, b):
    x = np.ascontiguousarray(np.asarray(Simple_vector, dtype=np.float32))
    labels = np.asarray(label_list)
    w_L = np.asarray(w_L, dtype=np.float32).reshape(1, C)
    w_H = np.asarray(w_H, dtype=np.float32).reshape(1, 1)
    b = np.asarray(b, dtype=np.float32).reshape(1, 1)

    n = x.shape[0]
    n_shard = n // N_CORES
    nc = _get_nc(n_shard, wl_is_ones=bool(np.all(w_L == 1.0)))

    in_maps = [
        {
            "x": x[i * n_shard : (i + 1) * n_shard],
            "w_L": w_L,
            "w_H": w_H,
            "b": b,
        }
        for i in range(N_CORES)
    ]
    res = run_bass_kernel_spmd(nc, in_maps, core_ids=list(range(N_CORES)))

    scaled = np.concatenate([np.asarray(r["scaled"]) for r in res.results], axis=0)
    n_tiles = n_shard // P
    logz_rows = np.concatenate(
        [np.asarray(r["logz"]).reshape(P, n_tiles).T.ravel() for r in res.results]
    )
    picked = np.take_along_axis(
        scaled, labels.astype(np.int64).reshape(-1, 1), axis=1
    )[:, 0]
    loss = np.float32((logz_rows.astype(np.float64) - picked.astype(np.float64)).mean())
    return scaled, loss
